# revision 17
# baseline (speedup 1.0000x reference)
"""Trainium2 Bass kernel for EuclideanCodebook (VQ) forward + EMA stats.

Math (forward values only; straight-through terms cancel in the forward pass):
  flatten = x.reshape(M, D)                     M=16384, D=64, K=4096
  dist[m,k]   = -(|x_m|^2 + |e_k|^2 - 2 x_m.e_k)          [output, 256MB]
  idx[m]      = argmax_k dist[m,k]
  embed_ind   = idx as float32
  quantize    = embed[idx]
  bins[k]     = |{m : idx[m]=k}|
  cluster_size_new = 0.1*cluster_size + 0.9*bins
  embed_sum[k,d]   = sum_{m: idx[m]=k} x[m,d]
  embed_avg_new    = 0.1*embed_avg + 0.9*embed_sum

Sharding: data-parallel over tokens, 2048 tokens/core on 8 cores; codebook
replicated. Per-core device program (SPMD, no collectives):
  - dist tile [128, 4096] per m-tile via PE matmul with e_sq folded into an
    extra contraction row (lhsT = [x^T; 1], rhs = [2*embed^T; -e_sq]).
  - ACT evicts PSUM->SBUF adding the per-token -|x|^2 bias -> final dist.
  - GPSIMD computes the row max.
  - One DVE scalar_tensor_tensor computes iotahot = (dist==max)*(k+1) with
    accum_out = idx+1 (the "+1" avoids the k=0 annihilation).
  - PE accumulates embed_sum^T(scaled) = x^T @ iotahot over all m-tiles in
    PSUM, packed two k-halves across the 128 output partitions.
Host: gathers shards, divides the scaled embed_sum by (k+1), bincounts idx
for bins, gathers quantize rows, applies the EMA updates, and sums the
per-core embed_sum partials (the DDP all_reduce equivalent).
"""

import sys

import numpy as np

for _p in ("/opt/trn_rl_repo", "/opt/pypackages"):
    if _p not in sys.path:
        sys.path.insert(0, _p)

import concourse.bass as bass
import concourse.tile as tile
from concourse import bacc, mybir
from concourse.bass_utils import run_bass_kernel_spmd

F32 = mybir.dt.float32

B, N, DIM = 8, 2048, 64
H, K = 1, 4096
DECAY = 0.1
NCORES = 8
M_LOC = (B * N) // NCORES          # 2048 tokens per core
MT = 128                           # tokens per m-tile (partition dim)
NMT = M_LOC // MT                  # 16 m-tiles
DAUG = DIM + 1                     # contraction dim with the e_sq fold row


def build_program(reps: int = 1) -> bass.Bass:
    """Build the per-core SPMD program. `reps` > 1 repeats the whole compute
    pipeline (same I/O) for differential wall-clock timing of the HW exec."""
    # Bacc (not raw Bass): its compile() runs generate_event_semaphores,
    # which splits multi-semaphore waits down to the 1-wait-per-instruction
    # hardware limit (raw Tile output fails walrus codegen otherwise).
    nc = bacc.Bacc("TRN2", target_bir_lowering=False, debug=False,
                   num_devices=NCORES)

    # Per-core inputs
    xT_aug = nc.dram_tensor("xT_aug", [DAUG, M_LOC], F32, kind="ExternalInput").ap()
    x_es = nc.dram_tensor("x_es", [M_LOC, DIM], F32, kind="ExternalInput").ap()
    embT_aug = nc.dram_tensor("embT_aug", [DAUG, K], F32, kind="ExternalInput").ap()
    xsq_neg = nc.dram_tensor("xsq_neg", [MT, NMT], F32, kind="ExternalInput").ap()
    iota1 = nc.dram_tensor("iota1", [MT, K], F32, kind="ExternalInput").ap()

    # Per-core outputs
    dist_o = nc.dram_tensor("dist_o", [M_LOC, K], F32, kind="ExternalOutput").ap()
    es_o = nc.dram_tensor("es_o", [MT, K // 2], F32, kind="ExternalOutput").ap()
    idx_o = nc.dram_tensor("idx_o", [MT, NMT], F32, kind="ExternalOutput").ap()

    with tile.TileContext(nc) as tc:
        with (
            tc.tile_pool(name="consts", bufs=1) as consts,
            tc.tile_pool(name="dist", bufs=3) as dist_pool,
            tc.tile_pool(name="ioh", bufs=2) as ioh_pool,
            tc.tile_pool(name="small", bufs=4) as small_pool,
            tc.tile_pool(name="espart", bufs=2) as espart_pool,
            tc.tile_pool(name="pdist", bufs=2, space="PSUM") as pdist_pool,
            tc.tile_pool(name="pes", bufs=1, space="PSUM") as pes_pool,
        ):
            # Resident constants
            embT_sb = consts.tile([DAUG, K], F32)
            nc.sync.dma_start(embT_sb[:], embT_aug[:])
            iota1_sb = consts.tile([MT, K], F32)
            nc.sync.dma_start(iota1_sb[:], iota1[:])
            xT_sb = consts.tile([DAUG, M_LOC], F32)
            nc.sync.dma_start(xT_sb[:], xT_aug[:])
            xes_sb = consts.tile([MT, NMT * DIM], F32)
            for i in range(NMT):
                nc.sync.dma_start(xes_sb[:, i * DIM:(i + 1) * DIM],
                                  x_es[i * MT:(i + 1) * MT, :])
            xsq_sb = consts.tile([MT, NMT], F32)
            nc.sync.dma_start(xsq_sb[:], xsq_neg[:])
            idx_sb = consts.tile([MT, NMT], F32)

            # Packed embed_sum accumulator in SBUF (GPSIMD-accumulated):
            #   partitions 0:64  -> d rows for k in [0, 2048)
            #   partitions 64:128-> d rows for k in [2048, 4096)
            es_acc = consts.tile([MT, K // 2], F32)

            CHUNK = 1024           # psum dist chunk (2 banks)
            NCHUNK = K // CHUNK    # 4

            for _rep in range(reps):
              nc.gpsimd.memset(es_acc[:], 0.0)
              for i in range(NMT):
                dist_s = dist_pool.tile([MT, K], F32)
                lhsT_i = xT_sb[:, i * MT:(i + 1) * MT]
                for q in range(NCHUNK):
                    pq = pdist_pool.tile([MT, CHUNK], F32)
                    for j in range(CHUNK // 512):
                        k0 = q * CHUNK + j * 512
                        nc.tensor.matmul(
                            pq[:, j * 512:(j + 1) * 512],
                            lhsT=lhsT_i,
                            rhs=embT_sb[:, k0:k0 + 512],
                            start=True, stop=True,
                        )
                    # PSUM -> SBUF eviction fused with the -|x|^2 bias
                    nc.scalar.activation(
                        dist_s[:, q * CHUNK:(q + 1) * CHUNK],
                        pq[:],
                        mybir.ActivationFunctionType.Identity,
                        bias=xsq_sb[:, i:i + 1],
                        scale=1.0,
                    )
                # dist tile to DRAM
                nc.sync.dma_start(dist_o[i * MT:(i + 1) * MT, :], dist_s[:])

                # Row max (DVE; free-dim reduce is DVE-only)
                dmax = small_pool.tile([MT, 1], F32, tag="dmax")
                nc.vector.reduce_max(dmax[:], dist_s[:],
                                     axis=mybir.AxisListType.X)

                # iotahot = (dist == max) * (k+1);  accum = idx+1
                ioh = ioh_pool.tile([MT, K], F32)
                nc.vector.scalar_tensor_tensor(
                    ioh[:],
                    dist_s[:],
                    dmax[:],
                    iota1_sb[:],
                    op0=mybir.AluOpType.is_equal,
                    op1=mybir.AluOpType.mult,
                    accum_out=idx_sb[:, i:i + 1],
                )

                # Per-m-tile embed_sum^T partial: x_tile^T @ iotahot, packed
                # two k-halves across the 128 PSUM partitions. Each matmul is
                # its own complete group (written exactly once per m-tile).
                xes_i = xes_sb[:, i * DIM:(i + 1) * DIM]
                pes_t = pes_pool.tile([MT, K // 2], F32)
                for h in range(2):
                    for j in range(4):
                        k0 = h * (K // 2) + j * 512
                        c0 = j * 512
                        nc.tensor.matmul(
                            pes_t[h * DIM:(h + 1) * DIM, c0:c0 + 512],
                            lhsT=xes_i,
                            rhs=ioh[:, k0:k0 + 512],
                            start=True, stop=True,
                        )
                es_part = espart_pool.tile([MT, K // 2], F32)
                nc.scalar.activation(
                    es_part[:], pes_t[:],
                    mybir.ActivationFunctionType.Identity,
                    bias=0.0, scale=1.0,
                )
                nc.gpsimd.tensor_add(es_acc[:], es_acc[:], es_part[:])

            # Final evictions
            nc.sync.dma_start(es_o[:], es_acc[:])
            nc.sync.dma_start(idx_o[:], idx_sb[:])

    nc.compile()
    return nc


_NC_CACHE = {}


def _get_nc(reps: int = 1):
    if reps not in _NC_CACHE:
        _NC_CACHE[reps] = build_program(reps)
    return _NC_CACHE[reps]


def make_in_maps(x: np.ndarray, embed: np.ndarray):
    """Host-side shard + layout prep. Returns list of per-core input dicts."""
    x = np.ascontiguousarray(np.asarray(x, dtype=np.float32).reshape(-1, DIM))
    e0 = np.asarray(embed, dtype=np.float32).reshape(K, DIM)

    e_sq = np.sum(e0 * e0, axis=-1, dtype=np.float32)          # [K]
    embT_aug = np.empty((DAUG, K), dtype=np.float32)
    embT_aug[:DIM] = (2.0 * e0).T
    embT_aug[DIM] = -e_sq

    iota1 = np.broadcast_to(
        np.arange(1, K + 1, dtype=np.float32)[None, :], (MT, K)
    ).copy()

    in_maps = []
    for c in range(NCORES):
        xc = x[c * M_LOC:(c + 1) * M_LOC]                       # [2048, 64]
        xT_aug = np.empty((DAUG, M_LOC), dtype=np.float32)
        xT_aug[:DIM] = xc.T
        xT_aug[DIM] = 1.0
        xsq = np.sum(xc * xc, axis=-1, dtype=np.float32)        # [2048]
        xsq_neg = np.ascontiguousarray(-xsq.reshape(NMT, MT).T) # [128, 16]
        in_maps.append({
            "xT_aug": np.ascontiguousarray(xT_aug),
            "x_es": np.ascontiguousarray(xc),
            "embT_aug": embT_aug,
            "xsq_neg": xsq_neg,
            "iota1": iota1,
        })
    return in_maps


def postprocess(results, x, embed, cluster_size, embed_avg):
    """Gather/unshard device outputs and apply the cheap EMA/gather glue."""
    x = np.asarray(x, dtype=np.float32)
    e0 = np.asarray(embed, dtype=np.float32).reshape(K, DIM)
    cluster_size = np.asarray(cluster_size, dtype=np.float32)
    embed_avg = np.asarray(embed_avg, dtype=np.float32)

    dist = np.empty((1, B * N, K), dtype=np.float32)
    idx1 = np.empty(B * N, dtype=np.float32)
    es_packed = np.zeros((MT, K // 2), dtype=np.float32)
    for c in range(NCORES):
        r = results[c]
        dist[0, c * M_LOC:(c + 1) * M_LOC] = r["dist_o"]
        # idx_o is [128, 16] with column i = m-tile i -> token t = i*128 + p
        idx1[c * M_LOC:(c + 1) * M_LOC] = r["idx_o"].T.reshape(-1)
        es_packed += r["es_o"]        # all-reduce over the data-parallel axis

    idx = idx1.astype(np.int64) - 1
    if idx.min() < 0 or idx.max() >= K:
        raise RuntimeError(
            f"device idx out of range [{idx.min()}, {idx.max()}] - "
            "tie/duplicate in argmax one-hot?")

    embed_ind = idx.astype(np.float32).reshape(1, B * N)
    quantize = e0[idx].reshape(B, N, DIM)

    bins = np.bincount(idx, minlength=K).astype(np.float32)     # [K]
    cluster_size_new = cluster_size * DECAY + bins[None, :] * (1.0 - DECAY)

    # unpack [d + 64*h, k - 2048*h] -> [64, 4096], undo the (k+1) scaling
    es_T = np.concatenate([es_packed[:DIM], es_packed[DIM:]], axis=1)  # [64, K]
    embed_sum = (es_T / np.arange(1, K + 1, dtype=np.float32)[None, :]).T
    embed_avg_new = embed_avg * DECAY + embed_sum[None] * (1.0 - DECAY)

    return quantize, embed_ind, dist, cluster_size_new, embed_avg_new


def run_device(x, embed, reps: int = 1):
    """Run the device program; returns the per-core results list."""
    nc = _get_nc(reps)
    in_maps = make_in_maps(x, embed)
    res = run_bass_kernel_spmd(nc, in_maps, core_ids=list(range(NCORES)),
                               trace=False)
    return res.results


def kernel(x, embed, cluster_size, embed_avg):
    results = run_device(x, embed)
    return postprocess(results, x, embed, cluster_size, embed_avg)


# revision 20
# speedup vs baseline: 933.6635x; 933.6635x over previous
"""Trainium2 Bass kernel for EuclideanCodebook (VQ) forward + EMA stats.

Math (forward values only; straight-through terms cancel in the forward pass):
  flatten = x.reshape(M, D)                     M=16384, D=64, K=4096
  dist[m,k]   = -(|x_m|^2 + |e_k|^2 - 2 x_m.e_k)          [output, 256MB]
  idx[m]      = argmax_k dist[m,k]
  embed_ind   = idx as float32
  quantize    = embed[idx]
  bins[k]     = |{m : idx[m]=k}|
  cluster_size_new = 0.1*cluster_size + 0.9*bins
  embed_sum[k,d]   = sum_{m: idx[m]=k} x[m,d]
  embed_avg_new    = 0.1*embed_avg + 0.9*embed_sum

Sharding: data-parallel over tokens, 2048 tokens/core on 8 cores; codebook
replicated. Per-core device program (SPMD, no collectives):
  - dist tile [128, 4096] per m-tile via PE matmul with e_sq folded into an
    extra contraction row (lhsT = [x^T; 1], rhs = [2*embed^T; -e_sq]).
  - ACT evicts PSUM->SBUF adding the per-token -|x|^2 bias -> final dist.
  - GPSIMD computes the row max.
  - One DVE scalar_tensor_tensor computes iotahot = (dist==max)*(k+1) with
    accum_out = idx+1 (the "+1" avoids the k=0 annihilation).
  - PE accumulates embed_sum^T(scaled) = x^T @ iotahot over all m-tiles in
    PSUM, packed two k-halves across the 128 output partitions.
Host: gathers shards, divides the scaled embed_sum by (k+1), bincounts idx
for bins, gathers quantize rows, applies the EMA updates, and sums the
per-core embed_sum partials (the DDP all_reduce equivalent).
"""

import sys

import numpy as np

for _p in ("/opt/trn_rl_repo", "/opt/pypackages"):
    if _p not in sys.path:
        sys.path.insert(0, _p)

import concourse.bass as bass
import concourse.tile as tile
from concourse import bacc, mybir
from concourse.bass_utils import run_bass_kernel_spmd

F32 = mybir.dt.float32

B, N, DIM = 8, 2048, 64
H, K = 1, 4096
DECAY = 0.1
NCORES = 8
M_LOC = (B * N) // NCORES          # 2048 tokens per core
MT = 128                           # tokens per m-tile (partition dim)
NMT = M_LOC // MT                  # 16 m-tiles
DAUG = DIM + 1                     # contraction dim with the e_sq fold row


def build_program(reps: int = 1) -> bass.Bass:
    """Build the per-core SPMD program. `reps` > 1 repeats the whole compute
    pipeline (same I/O) for differential wall-clock timing of the HW exec."""
    # Bacc (not raw Bass): its compile() runs generate_event_semaphores,
    # which splits multi-semaphore waits down to the 1-wait-per-instruction
    # hardware limit (raw Tile output fails walrus codegen otherwise).
    nc = bacc.Bacc("TRN2", target_bir_lowering=False, debug=False,
                   num_devices=NCORES)

    # Per-core inputs
    xT_aug = nc.dram_tensor("xT_aug", [DAUG, M_LOC], F32, kind="ExternalInput").ap()
    x_es = nc.dram_tensor("x_es", [M_LOC, DIM], F32, kind="ExternalInput").ap()
    embT_aug = nc.dram_tensor("embT_aug", [DAUG, K], F32, kind="ExternalInput").ap()
    xsq_neg = nc.dram_tensor("xsq_neg", [MT, NMT], F32, kind="ExternalInput").ap()
    iota1 = nc.dram_tensor("iota1", [MT, K], F32, kind="ExternalInput").ap()

    # Per-core outputs
    dist_o = nc.dram_tensor("dist_o", [M_LOC, K], F32, kind="ExternalOutput").ap()
    es_o = nc.dram_tensor("es_o", [MT, K // 2], F32, kind="ExternalOutput").ap()
    idx_o = nc.dram_tensor("idx_o", [MT, NMT], F32, kind="ExternalOutput").ap()

    with tile.TileContext(nc) as tc:
        with (
            tc.tile_pool(name="consts", bufs=1) as consts,
            tc.tile_pool(name="dist", bufs=3) as dist_pool,
            tc.tile_pool(name="ioh", bufs=2) as ioh_pool,
            tc.tile_pool(name="small", bufs=4) as small_pool,
            tc.tile_pool(name="espart", bufs=2) as espart_pool,
            tc.tile_pool(name="pdist", bufs=2, space="PSUM") as pdist_pool,
            tc.tile_pool(name="pes", bufs=1, space="PSUM") as pes_pool,
        ):
            # Resident constants
            embT_sb = consts.tile([DAUG, K], F32)
            nc.sync.dma_start(embT_sb[:], embT_aug[:])
            iota1_sb = consts.tile([MT, K], F32)
            nc.sync.dma_start(iota1_sb[:], iota1[:])
            xT_sb = consts.tile([DAUG, M_LOC], F32)
            nc.sync.dma_start(xT_sb[:], xT_aug[:])
            xes_sb = consts.tile([MT, NMT * DIM], F32)
            for i in range(NMT):
                nc.sync.dma_start(xes_sb[:, i * DIM:(i + 1) * DIM],
                                  x_es[i * MT:(i + 1) * MT, :])
            xsq_sb = consts.tile([MT, NMT], F32)
            nc.sync.dma_start(xsq_sb[:], xsq_neg[:])
            idx_sb = consts.tile([MT, NMT], F32)

            # Packed embed_sum accumulator in SBUF (GPSIMD-accumulated):
            #   partitions 0:64  -> d rows for k in [0, 2048)
            #   partitions 64:128-> d rows for k in [2048, 4096)
            es_acc = consts.tile([MT, K // 2], F32)

            CHUNK = 1024           # psum dist chunk (2 banks)
            NCHUNK = K // CHUNK    # 4

            for _rep in range(reps):
              nc.gpsimd.memset(es_acc[:], 0.0)
              for i in range(NMT):
                dist_s = dist_pool.tile([MT, K], F32)
                lhsT_i = xT_sb[:, i * MT:(i + 1) * MT]
                for q in range(NCHUNK):
                    pq = pdist_pool.tile([MT, CHUNK], F32)
                    for j in range(CHUNK // 512):
                        k0 = q * CHUNK + j * 512
                        nc.tensor.matmul(
                            pq[:, j * 512:(j + 1) * 512],
                            lhsT=lhsT_i,
                            rhs=embT_sb[:, k0:k0 + 512],
                            start=True, stop=True,
                        )
                    # PSUM -> SBUF eviction fused with the -|x|^2 bias
                    nc.scalar.activation(
                        dist_s[:, q * CHUNK:(q + 1) * CHUNK],
                        pq[:],
                        mybir.ActivationFunctionType.Identity,
                        bias=xsq_sb[:, i:i + 1],
                        scale=1.0,
                    )
                # dist tile to DRAM
                nc.sync.dma_start(dist_o[i * MT:(i + 1) * MT, :], dist_s[:])

                # Row max (DVE; free-dim reduce is DVE-only)
                dmax = small_pool.tile([MT, 1], F32, tag="dmax")
                nc.vector.reduce_max(dmax[:], dist_s[:],
                                     axis=mybir.AxisListType.X)

                # iotahot = (dist == max) * (k+1);  accum = idx+1
                ioh = ioh_pool.tile([MT, K], F32)
                nc.vector.scalar_tensor_tensor(
                    ioh[:],
                    dist_s[:],
                    dmax[:],
                    iota1_sb[:],
                    op0=mybir.AluOpType.is_equal,
                    op1=mybir.AluOpType.mult,
                    accum_out=idx_sb[:, i:i + 1],
                )

                # Per-m-tile embed_sum^T partial: x_tile^T @ iotahot, packed
                # two k-halves across the 128 PSUM partitions. Each matmul is
                # its own complete group (written exactly once per m-tile).
                xes_i = xes_sb[:, i * DIM:(i + 1) * DIM]
                pes_t = pes_pool.tile([MT, K // 2], F32)
                for h in range(2):
                    for j in range(4):
                        k0 = h * (K // 2) + j * 512
                        c0 = j * 512
                        nc.tensor.matmul(
                            pes_t[h * DIM:(h + 1) * DIM, c0:c0 + 512],
                            lhsT=xes_i,
                            rhs=ioh[:, k0:k0 + 512],
                            start=True, stop=True,
                        )
                es_part = espart_pool.tile([MT, K // 2], F32)
                nc.scalar.activation(
                    es_part[:], pes_t[:],
                    mybir.ActivationFunctionType.Identity,
                    bias=0.0, scale=1.0,
                )
                nc.gpsimd.tensor_add(es_acc[:], es_acc[:], es_part[:])

            # Final evictions
            nc.sync.dma_start(es_o[:], es_acc[:])
            nc.sync.dma_start(idx_o[:], idx_sb[:])

    nc.compile()
    return nc


_NC_CACHE = {}


def _get_nc(reps: int = 1):
    if reps not in _NC_CACHE:
        _NC_CACHE[reps] = build_program(reps)
    return _NC_CACHE[reps]


def make_in_maps(x: np.ndarray, embed: np.ndarray):
    """Host-side shard + layout prep. Returns list of per-core input dicts."""
    x = np.ascontiguousarray(np.asarray(x, dtype=np.float32).reshape(-1, DIM))
    e0 = np.asarray(embed, dtype=np.float32).reshape(K, DIM)

    e_sq = np.sum(e0 * e0, axis=-1, dtype=np.float32)          # [K]
    embT_aug = np.empty((DAUG, K), dtype=np.float32)
    embT_aug[:DIM] = (2.0 * e0).T
    embT_aug[DIM] = -e_sq

    iota1 = np.broadcast_to(
        np.arange(1, K + 1, dtype=np.float32)[None, :], (MT, K)
    ).copy()

    in_maps = []
    for c in range(NCORES):
        xc = x[c * M_LOC:(c + 1) * M_LOC]                       # [2048, 64]
        xT_aug = np.empty((DAUG, M_LOC), dtype=np.float32)
        xT_aug[:DIM] = xc.T
        xT_aug[DIM] = 1.0
        xsq = np.sum(xc * xc, axis=-1, dtype=np.float32)        # [2048]
        xsq_neg = np.ascontiguousarray(-xsq.reshape(NMT, MT).T) # [128, 16]
        in_maps.append({
            "xT_aug": np.ascontiguousarray(xT_aug),
            "x_es": np.ascontiguousarray(xc),
            "embT_aug": embT_aug,
            "xsq_neg": xsq_neg,
            "iota1": iota1,
        })
    return in_maps


def postprocess(results, x, embed, cluster_size, embed_avg):
    """Gather/unshard device outputs and apply the cheap EMA/gather glue."""
    x = np.asarray(x, dtype=np.float32)
    e0 = np.asarray(embed, dtype=np.float32).reshape(K, DIM)
    cluster_size = np.asarray(cluster_size, dtype=np.float32)
    embed_avg = np.asarray(embed_avg, dtype=np.float32)

    dist = np.empty((1, B * N, K), dtype=np.float32)
    idx1 = np.empty(B * N, dtype=np.float32)
    es_packed = np.zeros((MT, K // 2), dtype=np.float32)
    for c in range(NCORES):
        r = results[c]
        dist[0, c * M_LOC:(c + 1) * M_LOC] = r["dist_o"]
        # idx_o is [128, 16] with column i = m-tile i -> token t = i*128 + p
        idx1[c * M_LOC:(c + 1) * M_LOC] = r["idx_o"].T.reshape(-1)
        es_packed += r["es_o"]        # all-reduce over the data-parallel axis

    idx = idx1.astype(np.int64) - 1
    if idx.min() < 0 or idx.max() >= K:
        raise RuntimeError(
            f"device idx out of range [{idx.min()}, {idx.max()}] - "
            "tie/duplicate in argmax one-hot?")

    embed_ind = idx.astype(np.float32).reshape(1, B * N)
    quantize = e0[idx].reshape(B, N, DIM)

    bins = np.bincount(idx, minlength=K).astype(np.float32)     # [K]
    cluster_size_new = cluster_size * DECAY + bins[None, :] * (1.0 - DECAY)

    # unpack [d + 64*h, k - 2048*h] -> [64, 4096], undo the (k+1) scaling
    es_T = np.concatenate([es_packed[:DIM], es_packed[DIM:]], axis=1)  # [64, K]
    embed_sum = (es_T / np.arange(1, K + 1, dtype=np.float32)[None, :]).T
    embed_avg_new = embed_avg * DECAY + embed_sum[None] * (1.0 - DECAY)

    return quantize, embed_ind, dist, cluster_size_new, embed_avg_new


def run_device(x, embed, reps: int = 1):
    """Run the device program; returns the per-core results list."""
    nc = _get_nc(reps)
    in_maps = make_in_maps(x, embed)
    res = run_bass_kernel_spmd(nc, in_maps, core_ids=list(range(NCORES)),
                               trace=False)
    return res.results


def make_timed_runner(x, embed, reps: int = 1):
    """Build a repeat-callable executor with device-resident I/O.

    Mirrors bass2jax.run_bass_via_pjrt's multi-core path, but keeps the
    jitted executable and the input buffers alive, and feeds each call's
    donated output buffers back in as the next call's donated "zero"
    operands - so steady-state calls move no host data. Returns
    `step() -> seconds` (blocking wall time of one execution).
    """
    import jax
    import numpy as _np
    from jax.experimental.shard_map import shard_map
    from jax.sharding import Mesh, NamedSharding, PartitionSpec
    from concourse import mybir as _mb
    from concourse.bass2jax import (_bass_exec_p, install_neuronx_cc_hook,
                                    partition_id_tensor)

    install_neuronx_cc_hook()
    nc = _get_nc(reps)
    assert nc.dbg_addr is None
    partition_name = (nc.partition_id_tensor.name
                      if nc.partition_id_tensor else None)

    in_names, out_names, out_avals, zero_outs = [], [], [], []
    for alloc in nc.m.functions[0].allocations:
        if not isinstance(alloc, _mb.MemoryLocationSet):
            continue
        name = alloc.memorylocations[0].name
        if alloc.kind == "ExternalInput":
            in_names.append(name)
        elif alloc.kind == "ExternalOutput":
            dt = _mb.dt.np(alloc.dtype)
            out_avals.append(
                jax.core.ShapedArray(tuple(alloc.tensor_shape), dt))
            out_names.append(name)
            zero_outs.append(_np.zeros(tuple(alloc.tensor_shape), dt))

    if partition_name is not None:
        in_names = [n for n in in_names if n != partition_name]
    n_params = len(in_names)
    n_outs = len(out_names)
    all_in_names = in_names + out_names
    if partition_name is not None:
        all_in_names = all_in_names + [partition_name]
    donate = tuple(range(n_params, n_params + n_outs))

    def _body(*args):
        operands = list(args)
        if partition_name is not None:
            operands.append(partition_id_tensor())
        outs = _bass_exec_p.bind(
            *operands,
            out_avals=tuple(out_avals),
            in_names=tuple(all_in_names),
            out_names=tuple(out_names),
            lowering_input_output_aliases=(),
            sim_require_finite=True,
            sim_require_nnan=True,
            nc=nc,
        )
        return tuple(outs)

    devices = jax.devices()[:NCORES]
    mesh = Mesh(_np.asarray(devices), ("core",))
    in_specs = (PartitionSpec("core"),) * (n_params + n_outs)
    out_specs = (PartitionSpec("core"),) * n_outs
    sharded = jax.jit(
        shard_map(_body, mesh=mesh, in_specs=in_specs, out_specs=out_specs,
                  check_rep=False),
        donate_argnums=donate, keep_unused=True,
    )

    in_maps = make_in_maps(x, embed)
    sh = NamedSharding(mesh, PartitionSpec("core"))
    ins_dev = [
        jax.device_put(
            np.concatenate([np.asarray(in_maps[c][n]) for c in range(NCORES)],
                           axis=0), sh)
        for n in in_names
    ]
    zeros = [
        jax.device_put(
            np.zeros((NCORES * z.shape[0], *z.shape[1:]), z.dtype), sh)
        for z in zero_outs
    ]

    state = {"outs": sharded(*ins_dev, *zeros)}   # warm-up (compiles)
    jax.block_until_ready(state["outs"])

    def step():
        import time as _t
        t0 = _t.perf_counter()
        state["outs"] = sharded(*ins_dev, *state["outs"])
        jax.block_until_ready(state["outs"])
        return _t.perf_counter() - t0

    return step


def kernel(x, embed, cluster_size, embed_avg):
    results = run_device(x, embed)
    return postprocess(results, x, embed, cluster_size, embed_avg)
